# revision 33
# baseline (speedup 1.0000x reference)
"""GQA attention (S=2048, D=4096, H=32, G=8, DH=128) on 8 trn2 cores.

Sharding: core i owns query heads [4i, 4i+4) and KV group i (column shards
of Wq/Wk/Wv). After attention each core holds a normalized context slice
ctx_loc [128, 4, 512] ([dh, head, query]); a per-chunk AllGather assembles
the full context and each core computes its 512-column shard of the output
projection. The host concatenates the 8 column shards.

All activations are feature-major ([feature, seq]):
  qT_h = Wq_h^T @ x^T           (PE, accumulate over D tiles)
  RoPE: the half-swap runs as two SBUF->SBUF DMAs against a host-negated
        sin table (no PE matmul, no extra PSUM bank)
  s[t,q] block = kT_tile.T @ qT chunk      (scoresT layout)
  p    = exp(s/sqrt(DH) - 4)    (ACT; bias keeps p in fp16 range)
  den  = running DVE sum of p tiles; 1/den via DVE fast reciprocal
  ctxT = v_block.T @ p          (PE accumulate)
  out  = ctx_tile.T @ Wo_shard  (PE, per-chunk after its AllGather)

Schedule: the whole kernel is one dense PE stream. Phase B (attention,
scalar-engine heavy) is interleaved INTO phase A's projection matmuls of
the next chunk, and phase C's output-projection matmuls fill phase B's
exp-latency gaps in the late iterations:
    A(0) | A(1)+B(0) | A(2)+B(1) | A(3)+B(2) | B(3)+C(0)+C(1) | C(2)+C(3)
A PE idle window >3.4us re-throttles the PE clock to 1.2 GHz, so density
is worth ~2x on its own. PSUM budget (8 banks): 6 projection accumulators
(reused by phase C's output accumulators via the same tag) + 1 score bank
+ 1 ctx bank. All HBM traffic moves in ~1MB slabs from host-preswizzled
[128, kt, col] layouts (the sync queue serializes dma_starts at ~0.6us
each, so small DMAs are poison).
"""

import math
import sys

if "/opt/trn_rl_repo" not in sys.path:
    sys.path.insert(0, "/opt/trn_rl_repo")

import numpy as np

S, D, H, G, DH = 2048, 4096, 32, 8, 128
N_CORES = 8
HPC = H // N_CORES          # query heads per core (4)
FPC = HPC * DH              # context features per core (512)
QC = 512                    # query chunk (matmul free dim)
NQC = S // QC               # 4
TB = 128                    # key block
NTB = S // TB               # 16
NKT = D // 128              # contraction tiles over D (32)
NJ = QC // TB               # key blocks per query chunk (4)
KSLAB = 8                   # kt tiles per x DMA slab
NSLAB = NKT // KSLAB        # 4
INV_SQRT_DH = 1.0 / math.sqrt(DH)
EXP_BIAS = -4.0             # keeps exp() outputs inside fp16 range
NEG_BIAS = -60000.0         # fp16-representable; exp() underflows to 0

_CACHE = {}


def _build_program():
    import concourse.mybir as mybir
    import concourse.tile as tile
    from concourse import bacc

    f32 = mybir.dt.float32
    f16 = mybir.dt.float16
    EXP = mybir.ActivationFunctionType.Exp

    nc = bacc.Bacc("TRN2", target_bir_lowering=False, debug=False,
                   num_devices=N_CORES)

    # host-preswizzled layouts: [128, kt, col] so each DMA is one 3D slab
    xTr_d = nc.dram_tensor("xTr", [128, NKT, S], f16, kind="ExternalInput")
    wq_d = nc.dram_tensor("wq", [128, NKT, FPC], f16, kind="ExternalInput")
    wk_d = nc.dram_tensor("wk", [128, NKT, DH], f16, kind="ExternalInput")
    wv_d = nc.dram_tensor("wv", [128, NKT, DH], f16, kind="ExternalInput")
    wo_d = nc.dram_tensor("wo", [128, NKT, FPC], f16, kind="ExternalInput")
    cosT_d = nc.dram_tensor("cosT", [DH, S], f16, kind="ExternalInput")
    # sinM = sin with rows [0, DH/2) negated: rotate_half(q)*sin == qswap*sinM
    sinM_d = nc.dram_tensor("sinM", [DH, S], f16, kind="ExternalInput")
    # triangle mask for the one diagonal 128x128 sub-block of each key block
    tri_d = nc.dram_tensor("tri", [TB, TB], f16, kind="ExternalInput")
    ident_d = nc.dram_tensor("ident", [TB, TB], f16, kind="ExternalInput")
    onesc_d = nc.dram_tensor("onesc", [TB, 1], f16, kind="ExternalInput")
    onesr_d = nc.dram_tensor("onesr", [1, DH], f32, kind="ExternalInput")
    out_d = nc.dram_tensor("out", [S, FPC], f32, kind="ExternalOutput")

    with tile.TileContext(nc) as tc:
        with tc.tile_pool(name="dram", bufs=1, space="DRAM") as dram:
            ctx_loc = [dram.tile([128, HPC, QC], f16, name=f"ctx_loc{qc}",
                                 tag=f"cl{qc}") for qc in range(NQC)]
            ctx_all = [dram.tile([N_CORES, 128, HPC, QC], f16,
                                 name=f"ctx_all{qc}", tag=f"ca{qc}",
                                 addr_space="Shared") for qc in range(NQC)]

            with tc.tile_pool(name="res", bufs=1) as res, \
                 tc.tile_pool(name="str", bufs=1) as pS, \
                 tc.tile_pool(name="ps", bufs=1, space="PSUM") as ps:

                # tiles for tiny consts; their DMAs are emitted inside the
                # second A(0) unit so the critical wk/x/wq loads go first
                ident_sb = res.tile([TB, TB], f16, tag="ident", name="ident_sb")
                onesc_sb = res.tile([TB, 1], f16, tag="onesc", name="onesc_sb")
                onesr_sb = res.tile([1, DH], f32, tag="onesr", name="onesr_sb")
                ebias_sb = res.tile([128, 1], f32, tag="ebias", name="ebias_sb")
                nc.vector.memset(ebias_sb[:], EXP_BIAS)
                tri_sb = res.tile([TB, TB], f16, tag="tri", name="tri_sb")

                # wk/wv on the scalar DMA queue: overlaps the sync-queue
                # x/wq stream, so the first matmul starts ~4us earlier
                wk_sb = res.tile([128, NKT, DH], f16, tag="wk", name="wk_sb")
                nc.scalar.dma_start(out=wk_sb[:], in_=wk_d[:])
                wv_sb = res.tile([128, NKT, DH], f16, tag="wv", name="wv_sb")
                nc.scalar.dma_start(out=wv_sb[:], in_=wv_d[:])
                wq_sb = res.tile([128, NKT, FPC], f16, tag="wq", name="wq_sb")
                wo_sb = res.tile([128, NKT, FPC], f16, tag="wo", name="wo_sb")

                # per-chunk activation tiles (separate tiles so cross-chunk
                # writer/reader deps stay slice-exact)
                qT_sb = [[res.tile([128, QC], f16, tag=f"qT{h}_{c}",
                                   name=f"qT{h}_{c}") for c in range(NQC)]
                         for h in range(HPC)]
                kT_sb = [res.tile([128, QC], f16, tag=f"kT{c}", name=f"kT{c}")
                         for c in range(NQC)]
                v_sb = [res.tile([128, NJ, TB], f16, tag=f"v{c}",
                                 name=f"v{c}") for c in range(NQC)]

                # ============ phase A unit generator (one chunk) ===========
                def a_units(c):
                    """Yield closures; each emits 6 matmuls (one kt across
                    the 6 projections). Final units emit rope + v-evict."""
                    csl = slice(c * QC, (c + 1) * QC)
                    cos_c = pS.tile([DH, QC], f16, tag="cosc", bufs=2,
                                    name="cos_c")
                    sin_c = pS.tile([DH, QC], f16, tag="sinc", bufs=2,
                                    name="sin_c")
                    if c > 0:
                        nc.sync.dma_start(out=cos_c[:], in_=cosT_d[:, csl])
                        nc.sync.dma_start(out=sin_c[:], in_=sinM_d[:, csl])
                    k_ps = ps.tile([128, QC], f32, tag="acc", bufs=6,
                                   name="k_ps")
                    vT_ps = ps.tile([128, QC], f32, tag="acc", bufs=6,
                                    name="vT_ps")
                    q_ps = [ps.tile([128, QC], f32, tag="acc", bufs=6,
                                    name=f"q_ps{h}") for h in range(HPC)]
                    xt = [None]

                    def unit(kt):
                        sl, k = divmod(kt, KSLAB)
                        if k == 0:
                            xt[0] = pS.tile([128, KSLAB, QC], f16, tag="xs",
                                            bufs=2, name="xt")
                            nc.sync.dma_start(
                                out=xt[0][:],
                                in_=xTr_d[:, sl * KSLAB:(sl + 1) * KSLAB,
                                          csl])
                            if c == 0:
                                ks = slice(sl * KSLAB, (sl + 1) * KSLAB)
                                nc.sync.dma_start(out=wq_sb[:, ks, :],
                                                  in_=wq_d[:, ks, :])
                            if c == 0 and sl == 0:
                                nc.sync.dma_start(out=cos_c[:],
                                                  in_=cosT_d[:, csl])
                                nc.sync.dma_start(out=sin_c[:],
                                                  in_=sinM_d[:, csl])
                        if c == 0 and kt == 1:
                            nc.sync.dma_start(out=ident_sb[:], in_=ident_d[:])
                            nc.sync.dma_start(out=onesc_sb[:], in_=onesc_d[:])
                            nc.sync.dma_start(out=onesr_sb[:], in_=onesr_d[:])
                            nc.sync.dma_start(out=tri_sb[:], in_=tri_d[:])
                        st, sp = kt == 0, kt == NKT - 1
                        xk = xt[0][:, k, :]
                        nc.tensor.matmul(k_ps[:], wk_sb[:, kt, :], xk,
                                         start=st, stop=sp)
                        nc.tensor.matmul(vT_ps[:], wv_sb[:, kt, :], xk,
                                         start=st, stop=sp)
                        for h in range(HPC):
                            nc.tensor.matmul(q_ps[h][:],
                                             wq_sb[:, kt, h * DH:(h + 1) * DH],
                                             xk, start=st, stop=sp)

                    def rope(src_ps, dst_ap):
                        qc_sb = pS.tile([128, QC], f16, tag="ropecp", bufs=2,
                                        name="qc_sb")
                        nc.scalar.copy(qc_sb[:], src_ps[:])
                        qsw = pS.tile([128, QC], f16, tag="ropesw", bufs=2,
                                      name="qsw")
                        hf = DH // 2
                        nc.sync.dma_start(out=qsw[0:hf, :],
                                            in_=qc_sb[hf:DH, :])
                        nc.sync.dma_start(out=qsw[hf:DH, :],
                                            in_=qc_sb[0:hf, :])
                        t1 = pS.tile([128, QC], f16, tag="ropet1", bufs=2,
                                     name="t1")
                        nc.vector.tensor_mul(t1[:], qsw[:], sin_c[:])
                        nc.vector.tensor_mul(dst_ap, qc_sb[:], cos_c[:])
                        nc.vector.tensor_add(dst_ap, dst_ap, t1[:])

                    def tail_k():
                        rope(k_ps, kT_sb[c][:])

                    def tail_v():
                        vts = pS.tile([128, QC], f16, tag="vts", bufs=2,
                                      name="vts")
                        nc.scalar.copy(vts[:], vT_ps[:])
                        for sb in range(NJ):
                            tr_ps = ps.tile([TB, TB], f16, tag="s", bufs=1,
                                            name="tr_ps")
                            nc.tensor.transpose(tr_ps[:],
                                                vts[:, sb * TB:(sb + 1) * TB],
                                                ident_sb[:])
                            nc.scalar.copy(v_sb[c][:, sb, :], tr_ps[:])

                    for kt in range(NKT):
                        yield lambda kt=kt: unit(kt)
                    yield tail_k
                    yield tail_v
                    for h in range(HPC):
                        yield lambda h=h: rope(q_ps[h], qT_sb[h][c][:])

                # ================== phase B (one chunk) ====================
                def b_steps(qcn):
                    """Yield (step, kind) closures: single-head passes over
                    the key blocks; each block step takes a filler callable
                    run between its score and ctx matmuls."""
                    ntb = (qcn + 1) * NJ

                    def make_pass(h):
                        den = pS.tile([128, QC], f32, tag="den", bufs=2,
                                      name="den")
                        ctx_ps = ps.tile([128, QC], f32, tag="ctx", bufs=1,
                                         name="ctx_ps")
                        dr = [None]

                        def block(tb, filler):
                            j = tb - qcn * NJ
                            # diagonal key blocks: queries < j*TB are fully
                            # masked — skip their columns entirely
                            q0 = max(j, 0) * TB
                            w = QC - q0
                            s_ps = ps.tile([128, QC], f32, tag="s", bufs=1,
                                           name="s_ps")
                            nc.tensor.matmul(
                                s_ps[:, :w],
                                kT_sb[tb // NJ][:, (tb % NJ) * TB:
                                                (tb % NJ + 1) * TB],
                                qT_sb[h][qcn][:, q0:], start=True, stop=True)
                            if j >= 0:
                                # only the leading TB columns of the live
                                # range form the triangle
                                nc.vector.tensor_add(s_ps[:, :TB],
                                                     s_ps[:, :TB], tri_sb[:])
                            p_sb = pS.tile([128, QC], f16, tag="p", bufs=8,
                                           name="p_sb")
                            nc.scalar.activation(p_sb[:, :w], s_ps[:, :w],
                                                 EXP, bias=ebias_sb[:],
                                                 scale=INV_SQRT_DH)
                            if tb == 0:
                                nc.vector.tensor_copy(den[:], p_sb[:])
                            elif tb == ntb - 1:
                                # last block is diagonal j=NJ-1 (w == TB)
                                d = pS.tile([128, QC], f16, tag="dr", bufs=2,
                                            name="dr")
                                nc.vector.tensor_copy(d[:, :q0], den[:, :q0])
                                nc.vector.tensor_add(d[:, q0:], den[:, q0:],
                                                     p_sb[:, :w])
                                dr[0] = d
                            else:
                                nc.vector.tensor_add(den[:, q0:], den[:, q0:],
                                                     p_sb[:, :w])
                            filler()
                            nc.tensor.matmul(ctx_ps[:, q0:],
                                             v_sb[tb // NJ][:, tb % NJ, :],
                                             p_sb[:, :w], start=(tb == 0),
                                             stop=(tb == ntb - 1))

                        def normalize(filler):
                            aux1 = ps.tile([128, QC], f32, tag="s", bufs=1,
                                           name="aux1")
                            nc.tensor.matmul(aux1[:1, :], onesc_sb[:],
                                             dr[0][:], start=True, stop=True)
                            recf = pS.tile([1, QC], f32, tag="recf", bufs=2,
                                           name="recf")
                            nc.vector.reciprocal_approx_fast(out=recf[:],
                                                             in_=aux1[:1, :])
                            filler()
                            aux2 = ps.tile([128, QC], f32, tag="s", bufs=1,
                                           name="aux2")
                            nc.tensor.matmul(aux2[:], onesr_sb[:], recf[:],
                                             start=True, stop=True)
                            rb = pS.tile([128, QC], f16, tag="rb", bufs=2,
                                         name="rb")
                            nc.vector.tensor_copy(rb[:], aux2[:])
                            ctx_sb = pS.tile([128, QC], f16, tag="ctxsb",
                                             bufs=2, name="ctx_sb")
                            nc.vector.tensor_mul(ctx_sb[:], ctx_ps[:], rb[:])
                            # scalar-queue DMA: the sync queue gets dammed
                            # behind the previous AllGather (ring-hazard
                            # wait), which would delay this write and with it
                            # the next AllGather's trigger
                            nc.scalar.dma_start(out=ctx_loc[qcn][:, h, :],
                                                in_=ctx_sb[:])

                        for tb in range(ntb):
                            yield lambda filler, tb=tb: block(tb, filler)
                        yield normalize

                    for h in range(HPC):
                        yield from make_pass(h)

                    def trigger(filler):
                        filler()
                        nc.gpsimd.collective_compute(
                            "AllGather", mybir.AluOpType.bypass,
                            replica_groups=[list(range(N_CORES))],
                            ins=[ctx_loc[qcn].opt()],
                            outs=[ctx_all[qcn].opt()])
                    yield trigger

                # ================== phase C (one chunk) ====================
                def c_steps(qcn):
                    ct = [None] * N_CORES

                    def load_half(half):
                        for i in range(N_CORES):
                            t = pS.tile([128, HPC, QC // 2], f16, tag="ct",
                                        bufs=9, name="ct")
                            nc.sync.dma_start(
                                out=t[:],
                                in_=ctx_all[qcn][i][:, :,
                                                    half * (QC // 2):
                                                    (half + 1) * (QC // 2)])
                            ct[i] = t

                    o_ps = [None]
                    o_cnt = [0]

                    def mm_run(qb, i0):
                        if o_cnt[0] == 0:
                            o_ps[0] = ps.tile([TB, FPC], f32, tag="acc",
                                              bufs=6, name="o_ps")
                        qoff = (qb % 2) * TB
                        for i in (i0, i0 + 1):
                            for jj in range(HPC):
                                kt = i * HPC + jj
                                nc.tensor.matmul(
                                    o_ps[0][:], ct[i][:, jj, qoff:qoff + TB],
                                    wo_sb[:, kt, :], start=(kt == 0),
                                    stop=(kt == NKT - 1))
                        o_cnt[0] += 2
                        if o_cnt[0] == N_CORES:
                            o_cnt[0] = 0
                            o_sb = pS.tile([TB, FPC], f32, tag="osb", bufs=2,
                                           name="o_sb")
                            nc.vector.tensor_copy(o_sb[:], o_ps[0][:])
                            qrow = qcn * QC + qb * TB
                            nc.scalar.dma_start(out=out_d[qrow:qrow + TB, :],
                                                in_=o_sb[:])

                    for qb in range(NJ):
                        if qb % 2 == 0:
                            yield lambda h=qb // 2: load_half(h)
                        for i0 in (0, 2, 4, 6):
                            yield lambda qb=qb, i0=i0: mm_run(qb, i0)

                # =================== interleaved emission ==================
                def emit(b_gen, fill_steps):
                    """Emit B steps, injecting filler closures into the
                    exp-latency slots, spread evenly (exact Bresenham)."""
                    fill = list(fill_steps)
                    bs = list(b_gen) if b_gen is not None else []
                    fi = [0]
                    nf, nb = len(fill), len(bs)

                    def filler_n(n):
                        def f():
                            for _ in range(n):
                                if fi[0] < nf:
                                    fill[fi[0]]()
                                    fi[0] += 1
                        return f

                    for bi, bstep in enumerate(bs):
                        n = (bi + 1) * nf // nb - bi * nf // nb
                        bstep(filler_n(n))
                    while fi[0] < nf:
                        fill[fi[0]]()
                        fi[0] += 1

                def as_fill(units):
                    # adapt no-arg closures to filler-taking b-steps
                    return [(lambda f, u=u: (u(), f())) for u in units]

                # A(0) runs alone (nothing to overlap yet)
                emit(as_fill(a_units(0)), [])
                # wo needed from C(0); loads behind the later A chunks
                for sl in range(NSLAB):
                    ks = slice(sl * KSLAB, (sl + 1) * KSLAB)
                    nc.sync.dma_start(out=wo_sb[:, ks, :], in_=wo_d[:, ks, :])

                emit(b_steps(0), a_units(1))
                emit(b_steps(1), a_units(2))
                emit(b_steps(2), a_units(3))
                emit(b_steps(3), list(c_steps(0)) + list(c_steps(1)))
                # trailing C(2) covers AllGather(3)'s latency; C(3) then
                # starts dense
                emit(None, c_steps(2))
                emit(None, c_steps(3))
    nc.compile()
    return nc


def _host_consts():
    ident = np.eye(TB, dtype=np.float16)
    onesc = np.ones((TB, 1), dtype=np.float16)
    onesr = np.ones((1, DH), dtype=np.float32)
    tloc = np.arange(TB)[:, None]
    qloc = np.arange(TB)[None, :]
    tri = np.where(tloc <= qloc, 0.0, NEG_BIAS).astype(np.float16)
    return ident, onesc, onesr, tri


def _swizzle(w):
    # [D, C] -> [128, NKT, C] with element (p, kt, c) = w[kt*128 + p, c]
    return np.ascontiguousarray(
        w.reshape(NKT, 128, w.shape[1]).transpose(1, 0, 2)).astype(np.float16)


def kernel(x, mask, cos, sin, Wq, Wk, Wv, Wo):
    from concourse.bass_utils import run_bass_kernel_spmd

    if "nc" not in _CACHE:
        _CACHE["nc"] = _build_program()
    nc = _CACHE["nc"]

    x = np.asarray(x, dtype=np.float32)
    cos = np.asarray(cos, dtype=np.float32)
    sin = np.asarray(sin, dtype=np.float32)
    Wq = np.asarray(Wq, dtype=np.float32)
    Wk = np.asarray(Wk, dtype=np.float32)
    Wv = np.asarray(Wv, dtype=np.float32)
    Wo = np.asarray(Wo, dtype=np.float32)

    xTr = _swizzle(np.ascontiguousarray(x[0].T))       # [128, NKT, S]
    cosT = np.ascontiguousarray(cos.T).astype(np.float16)
    sinM = np.ascontiguousarray(sin.T).astype(np.float16)
    sinM[:DH // 2] = -sinM[:DH // 2]
    ident, onesc, onesr, tri = _host_consts()

    in_maps = []
    for i in range(N_CORES):
        in_maps.append({
            "xTr": xTr,
            "wq": _swizzle(Wq[:, i * FPC:(i + 1) * FPC]),
            "wk": _swizzle(Wk[:, i * DH:(i + 1) * DH]),
            "wv": _swizzle(Wv[:, i * DH:(i + 1) * DH]),
            "wo": _swizzle(Wo[:, i * FPC:(i + 1) * FPC]),
            "cosT": cosT,
            "sinM": sinM,
            "tri": tri,
            "ident": ident,
            "onesc": onesc,
            "onesr": onesr,
        })

    import os
    trace = bool(os.environ.get("BASS_TRACE"))
    res = run_bass_kernel_spmd(nc, in_maps, list(range(N_CORES)), trace=trace)
    _CACHE["last_exec_time_ns"] = res.exec_time_ns

    out = np.concatenate([res.results[i]["out"] for i in range(N_CORES)],
                         axis=1)
    return out[None]


# revision 34
# speedup vs baseline: 1.0065x; 1.0065x over previous
"""GQA attention (S=2048, D=4096, H=32, G=8, DH=128) on 8 trn2 cores.

Sharding: core i owns query heads [4i, 4i+4) and KV group i (column shards
of Wq/Wk/Wv). After attention each core holds a normalized context slice
ctx_loc [128, 4, 512] ([dh, head, query]); a per-chunk AllGather assembles
the full context and each core computes its 512-column shard of the output
projection. The host concatenates the 8 column shards.

All activations are feature-major ([feature, seq]):
  qT_h = Wq_h^T @ x^T           (PE, accumulate over D tiles)
  RoPE: the half-swap runs as two SBUF->SBUF DMAs against a host-negated
        sin table (no PE matmul, no extra PSUM bank)
  s[t,q] block = kT_tile.T @ qT chunk      (scoresT layout)
  p    = exp(s/sqrt(DH) - 4)    (ACT; bias keeps p in fp16 range)
  den  = running DVE sum of p tiles; 1/den via DVE fast reciprocal
  ctxT = v_block.T @ p          (PE accumulate)
  out  = ctx_tile.T @ Wo_shard  (PE, per-chunk after its AllGather)

Schedule: the whole kernel is one dense PE stream. Phase B (attention,
scalar-engine heavy) is interleaved INTO phase A's projection matmuls of
the next chunk, and phase C's output-projection matmuls fill phase B's
exp-latency gaps in the late iterations:
    A(0) | A(1)+B(0) | A(2)+B(1) | A(3)+B(2) | B(3)+C(0)+C(1) | C(2)+C(3)
A PE idle window >3.4us re-throttles the PE clock to 1.2 GHz, so density
is worth ~2x on its own. PSUM budget (8 banks): 6 projection accumulators
(reused by phase C's output accumulators via the same tag) + 1 score bank
+ 1 ctx bank. All HBM traffic moves in ~1MB slabs from host-preswizzled
[128, kt, col] layouts (the sync queue serializes dma_starts at ~0.6us
each, so small DMAs are poison).
"""

import math
import sys

if "/opt/trn_rl_repo" not in sys.path:
    sys.path.insert(0, "/opt/trn_rl_repo")

import numpy as np

S, D, H, G, DH = 2048, 4096, 32, 8, 128
N_CORES = 8
HPC = H // N_CORES          # query heads per core (4)
FPC = HPC * DH              # context features per core (512)
QC = 512                    # query chunk (matmul free dim)
NQC = S // QC               # 4
TB = 128                    # key block
NTB = S // TB               # 16
NKT = D // 128              # contraction tiles over D (32)
NJ = QC // TB               # key blocks per query chunk (4)
KSLAB = 8                   # kt tiles per x DMA slab
NSLAB = NKT // KSLAB        # 4
INV_SQRT_DH = 1.0 / math.sqrt(DH)
EXP_BIAS = -4.0             # keeps exp() outputs inside fp16 range
NEG_BIAS = -60000.0         # fp16-representable; exp() underflows to 0

_CACHE = {}


def _build_program():
    import concourse.mybir as mybir
    import concourse.tile as tile
    from concourse import bacc

    f32 = mybir.dt.float32
    f16 = mybir.dt.float16
    EXP = mybir.ActivationFunctionType.Exp

    nc = bacc.Bacc("TRN2", target_bir_lowering=False, debug=False,
                   num_devices=N_CORES)

    # host-preswizzled layouts: [128, kt, col] so each DMA is one 3D slab
    xTr_d = nc.dram_tensor("xTr", [128, NKT, S], f16, kind="ExternalInput")
    wq_d = nc.dram_tensor("wq", [128, NKT, FPC], f16, kind="ExternalInput")
    wk_d = nc.dram_tensor("wk", [128, NKT, DH], f16, kind="ExternalInput")
    wv_d = nc.dram_tensor("wv", [128, NKT, DH], f16, kind="ExternalInput")
    wo_d = nc.dram_tensor("wo", [128, NKT, FPC], f16, kind="ExternalInput")
    cosT_d = nc.dram_tensor("cosT", [DH, S], f16, kind="ExternalInput")
    # sinM = sin with rows [0, DH/2) negated: rotate_half(q)*sin == qswap*sinM
    sinM_d = nc.dram_tensor("sinM", [DH, S], f16, kind="ExternalInput")
    # triangle mask for the one diagonal 128x128 sub-block of each key block
    tri_d = nc.dram_tensor("tri", [TB, TB], f16, kind="ExternalInput")
    ident_d = nc.dram_tensor("ident", [TB, TB], f16, kind="ExternalInput")
    onesc_d = nc.dram_tensor("onesc", [TB, 1], f16, kind="ExternalInput")
    onesr_d = nc.dram_tensor("onesr", [1, DH], f32, kind="ExternalInput")
    out_d = nc.dram_tensor("out", [S, FPC], f32, kind="ExternalOutput")

    with tile.TileContext(nc) as tc:
        with tc.tile_pool(name="dram", bufs=1, space="DRAM") as dram:
            ctx_loc = [dram.tile([128, HPC, QC], f16, name=f"ctx_loc{qc}",
                                 tag=f"cl{qc}") for qc in range(NQC)]
            ctx_all = [dram.tile([N_CORES, 128, HPC, QC], f16,
                                 name=f"ctx_all{qc}", tag=f"ca{qc}",
                                 addr_space="Shared") for qc in range(NQC)]

            with tc.tile_pool(name="res", bufs=1) as res, \
                 tc.tile_pool(name="str", bufs=1) as pS, \
                 tc.tile_pool(name="ps", bufs=1, space="PSUM") as ps:

                # tiles for tiny consts; their DMAs are emitted inside the
                # second A(0) unit so the critical wk/x/wq loads go first
                ident_sb = res.tile([TB, TB], f16, tag="ident", name="ident_sb")
                onesc_sb = res.tile([TB, 1], f16, tag="onesc", name="onesc_sb")
                onesr_sb = res.tile([1, DH], f32, tag="onesr", name="onesr_sb")
                ebias_sb = res.tile([128, 1], f32, tag="ebias", name="ebias_sb")
                nc.vector.memset(ebias_sb[:], EXP_BIAS)
                tri_sb = res.tile([TB, TB], f16, tag="tri", name="tri_sb")

                wk_sb = res.tile([128, NKT, DH], f16, tag="wk", name="wk_sb")
                nc.sync.dma_start(out=wk_sb[:], in_=wk_d[:])
                wv_sb = res.tile([128, NKT, DH], f16, tag="wv", name="wv_sb")
                nc.sync.dma_start(out=wv_sb[:], in_=wv_d[:])
                wq_sb = res.tile([128, NKT, FPC], f16, tag="wq", name="wq_sb")
                wo_sb = res.tile([128, NKT, FPC], f16, tag="wo", name="wo_sb")

                # per-chunk activation tiles (separate tiles so cross-chunk
                # writer/reader deps stay slice-exact)
                qT_sb = [[res.tile([128, QC], f16, tag=f"qT{h}_{c}",
                                   name=f"qT{h}_{c}") for c in range(NQC)]
                         for h in range(HPC)]
                kT_sb = [res.tile([128, QC], f16, tag=f"kT{c}", name=f"kT{c}")
                         for c in range(NQC)]
                v_sb = [res.tile([128, NJ, TB], f16, tag=f"v{c}",
                                 name=f"v{c}") for c in range(NQC)]

                # ============ phase A unit generator (one chunk) ===========
                def a_units(c):
                    """Yield closures; each emits 6 matmuls (one kt across
                    the 6 projections). Final units emit rope + v-evict."""
                    csl = slice(c * QC, (c + 1) * QC)
                    cos_c = pS.tile([DH, QC], f16, tag="cosc", bufs=2,
                                    name="cos_c")
                    sin_c = pS.tile([DH, QC], f16, tag="sinc", bufs=2,
                                    name="sin_c")
                    if c > 0:
                        nc.sync.dma_start(out=cos_c[:], in_=cosT_d[:, csl])
                        nc.sync.dma_start(out=sin_c[:], in_=sinM_d[:, csl])
                    k_ps = ps.tile([128, QC], f32, tag="acc", bufs=6,
                                   name="k_ps")
                    vT_ps = ps.tile([128, QC], f32, tag="acc", bufs=6,
                                    name="vT_ps")
                    q_ps = [ps.tile([128, QC], f32, tag="acc", bufs=6,
                                    name=f"q_ps{h}") for h in range(HPC)]
                    xt = [None]

                    def unit(kt):
                        sl, k = divmod(kt, KSLAB)
                        if k == 0:
                            xt[0] = pS.tile([128, KSLAB, QC], f16, tag="xs",
                                            bufs=2, name="xt")
                            nc.sync.dma_start(
                                out=xt[0][:],
                                in_=xTr_d[:, sl * KSLAB:(sl + 1) * KSLAB,
                                          csl])
                            if c == 0:
                                ks = slice(sl * KSLAB, (sl + 1) * KSLAB)
                                nc.sync.dma_start(out=wq_sb[:, ks, :],
                                                  in_=wq_d[:, ks, :])
                            if c == 0 and sl == 0:
                                nc.sync.dma_start(out=cos_c[:],
                                                  in_=cosT_d[:, csl])
                                nc.sync.dma_start(out=sin_c[:],
                                                  in_=sinM_d[:, csl])
                        if c == 0 and kt == 1:
                            nc.sync.dma_start(out=ident_sb[:], in_=ident_d[:])
                            nc.sync.dma_start(out=onesc_sb[:], in_=onesc_d[:])
                            nc.sync.dma_start(out=onesr_sb[:], in_=onesr_d[:])
                            nc.sync.dma_start(out=tri_sb[:], in_=tri_d[:])
                        st, sp = kt == 0, kt == NKT - 1
                        xk = xt[0][:, k, :]
                        nc.tensor.matmul(k_ps[:], wk_sb[:, kt, :], xk,
                                         start=st, stop=sp)
                        nc.tensor.matmul(vT_ps[:], wv_sb[:, kt, :], xk,
                                         start=st, stop=sp)
                        for h in range(HPC):
                            nc.tensor.matmul(q_ps[h][:],
                                             wq_sb[:, kt, h * DH:(h + 1) * DH],
                                             xk, start=st, stop=sp)

                    def rope(src_ps, dst_ap):
                        qc_sb = pS.tile([128, QC], f16, tag="ropecp", bufs=2,
                                        name="qc_sb")
                        nc.scalar.copy(qc_sb[:], src_ps[:])
                        qsw = pS.tile([128, QC], f16, tag="ropesw", bufs=2,
                                      name="qsw")
                        hf = DH // 2
                        nc.sync.dma_start(out=qsw[0:hf, :],
                                            in_=qc_sb[hf:DH, :])
                        nc.sync.dma_start(out=qsw[hf:DH, :],
                                            in_=qc_sb[0:hf, :])
                        t1 = pS.tile([128, QC], f16, tag="ropet1", bufs=2,
                                     name="t1")
                        nc.vector.tensor_mul(t1[:], qsw[:], sin_c[:])
                        nc.vector.tensor_mul(dst_ap, qc_sb[:], cos_c[:])
                        nc.vector.tensor_add(dst_ap, dst_ap, t1[:])

                    def tail_k():
                        rope(k_ps, kT_sb[c][:])

                    def tail_v():
                        vts = pS.tile([128, QC], f16, tag="vts", bufs=2,
                                      name="vts")
                        nc.scalar.copy(vts[:], vT_ps[:])
                        for sb in range(NJ):
                            tr_ps = ps.tile([TB, TB], f16, tag="s", bufs=1,
                                            name="tr_ps")
                            nc.tensor.transpose(tr_ps[:],
                                                vts[:, sb * TB:(sb + 1) * TB],
                                                ident_sb[:])
                            nc.scalar.copy(v_sb[c][:, sb, :], tr_ps[:])

                    for kt in range(NKT):
                        yield lambda kt=kt: unit(kt)
                    yield tail_k
                    yield tail_v
                    for h in range(HPC):
                        yield lambda h=h: rope(q_ps[h], qT_sb[h][c][:])

                # ================== phase B (one chunk) ====================
                def b_steps(qcn):
                    """Yield (step, kind) closures: single-head passes over
                    the key blocks; each block step takes a filler callable
                    run between its score and ctx matmuls."""
                    ntb = (qcn + 1) * NJ

                    def make_pass(h):
                        den = pS.tile([128, QC], f32, tag="den", bufs=2,
                                      name="den")
                        ctx_ps = ps.tile([128, QC], f32, tag="ctx", bufs=1,
                                         name="ctx_ps")
                        dr = [None]

                        def block(tb, filler):
                            j = tb - qcn * NJ
                            # diagonal key blocks: queries < j*TB are fully
                            # masked — skip their columns entirely
                            q0 = max(j, 0) * TB
                            w = QC - q0
                            s_ps = ps.tile([128, QC], f32, tag="s", bufs=1,
                                           name="s_ps")
                            nc.tensor.matmul(
                                s_ps[:, :w],
                                kT_sb[tb // NJ][:, (tb % NJ) * TB:
                                                (tb % NJ + 1) * TB],
                                qT_sb[h][qcn][:, q0:], start=True, stop=True)
                            if j >= 0:
                                # only the leading TB columns of the live
                                # range form the triangle
                                nc.vector.tensor_add(s_ps[:, :TB],
                                                     s_ps[:, :TB], tri_sb[:])
                            p_sb = pS.tile([128, QC], f16, tag="p", bufs=8,
                                           name="p_sb")
                            nc.scalar.activation(p_sb[:, :w], s_ps[:, :w],
                                                 EXP, bias=ebias_sb[:],
                                                 scale=INV_SQRT_DH)
                            if tb == 0:
                                nc.vector.tensor_copy(den[:], p_sb[:])
                            elif tb == ntb - 1:
                                # last block is diagonal j=NJ-1 (w == TB)
                                d = pS.tile([128, QC], f16, tag="dr", bufs=2,
                                            name="dr")
                                nc.vector.tensor_copy(d[:, :q0], den[:, :q0])
                                nc.vector.tensor_add(d[:, q0:], den[:, q0:],
                                                     p_sb[:, :w])
                                dr[0] = d
                            else:
                                nc.vector.tensor_add(den[:, q0:], den[:, q0:],
                                                     p_sb[:, :w])
                            filler()
                            nc.tensor.matmul(ctx_ps[:, q0:],
                                             v_sb[tb // NJ][:, tb % NJ, :],
                                             p_sb[:, :w], start=(tb == 0),
                                             stop=(tb == ntb - 1))

                        def normalize(filler):
                            aux1 = ps.tile([128, QC], f32, tag="s", bufs=1,
                                           name="aux1")
                            nc.tensor.matmul(aux1[:1, :], onesc_sb[:],
                                             dr[0][:], start=True, stop=True)
                            recf = pS.tile([1, QC], f32, tag="recf", bufs=2,
                                           name="recf")
                            nc.vector.reciprocal_approx_fast(out=recf[:],
                                                             in_=aux1[:1, :])
                            filler()
                            aux2 = ps.tile([128, QC], f32, tag="s", bufs=1,
                                           name="aux2")
                            nc.tensor.matmul(aux2[:], onesr_sb[:], recf[:],
                                             start=True, stop=True)
                            rb = pS.tile([128, QC], f16, tag="rb", bufs=2,
                                         name="rb")
                            nc.vector.tensor_copy(rb[:], aux2[:])
                            ctx_sb = pS.tile([128, QC], f16, tag="ctxsb",
                                             bufs=2, name="ctx_sb")
                            nc.vector.tensor_mul(ctx_sb[:], ctx_ps[:], rb[:])
                            # scalar-queue DMA: the sync queue gets dammed
                            # behind the previous AllGather (ring-hazard
                            # wait), which would delay this write and with it
                            # the next AllGather's trigger
                            nc.scalar.dma_start(out=ctx_loc[qcn][:, h, :],
                                                in_=ctx_sb[:])

                        for tb in range(ntb):
                            yield lambda filler, tb=tb: block(tb, filler)
                        yield normalize

                    for h in range(HPC):
                        yield from make_pass(h)

                    def trigger(filler):
                        filler()
                        nc.gpsimd.collective_compute(
                            "AllGather", mybir.AluOpType.bypass,
                            replica_groups=[list(range(N_CORES))],
                            ins=[ctx_loc[qcn].opt()],
                            outs=[ctx_all[qcn].opt()])
                    yield trigger

                # ================== phase C (one chunk) ====================
                def c_steps(qcn):
                    ct = [None] * N_CORES

                    def load_half(half):
                        for i in range(N_CORES):
                            t = pS.tile([128, HPC, QC // 2], f16, tag="ct",
                                        bufs=9, name="ct")
                            nc.sync.dma_start(
                                out=t[:],
                                in_=ctx_all[qcn][i][:, :,
                                                    half * (QC // 2):
                                                    (half + 1) * (QC // 2)])
                            ct[i] = t

                    o_ps = [None]
                    o_cnt = [0]

                    def mm_run(qb, i0):
                        if o_cnt[0] == 0:
                            o_ps[0] = ps.tile([TB, FPC], f32, tag="acc",
                                              bufs=6, name="o_ps")
                        qoff = (qb % 2) * TB
                        for i in (i0, i0 + 1):
                            for jj in range(HPC):
                                kt = i * HPC + jj
                                nc.tensor.matmul(
                                    o_ps[0][:], ct[i][:, jj, qoff:qoff + TB],
                                    wo_sb[:, kt, :], start=(kt == 0),
                                    stop=(kt == NKT - 1))
                        o_cnt[0] += 2
                        if o_cnt[0] == N_CORES:
                            o_cnt[0] = 0
                            o_sb = pS.tile([TB, FPC], f32, tag="osb", bufs=2,
                                           name="o_sb")
                            nc.vector.tensor_copy(o_sb[:], o_ps[0][:])
                            qrow = qcn * QC + qb * TB
                            nc.sync.dma_start(out=out_d[qrow:qrow + TB, :],
                                                in_=o_sb[:])

                    for qb in range(NJ):
                        if qb % 2 == 0:
                            yield lambda h=qb // 2: load_half(h)
                        for i0 in (0, 2, 4, 6):
                            yield lambda qb=qb, i0=i0: mm_run(qb, i0)

                # =================== interleaved emission ==================
                def emit(b_gen, fill_steps):
                    """Emit B steps, injecting filler closures into the
                    exp-latency slots, spread evenly (exact Bresenham)."""
                    fill = list(fill_steps)
                    bs = list(b_gen) if b_gen is not None else []
                    fi = [0]
                    nf, nb = len(fill), len(bs)

                    def filler_n(n):
                        def f():
                            for _ in range(n):
                                if fi[0] < nf:
                                    fill[fi[0]]()
                                    fi[0] += 1
                        return f

                    for bi, bstep in enumerate(bs):
                        n = (bi + 1) * nf // nb - bi * nf // nb
                        bstep(filler_n(n))
                    while fi[0] < nf:
                        fill[fi[0]]()
                        fi[0] += 1

                def as_fill(units):
                    # adapt no-arg closures to filler-taking b-steps
                    return [(lambda f, u=u: (u(), f())) for u in units]

                # A(0) runs alone (nothing to overlap yet)
                emit(as_fill(a_units(0)), [])
                # wo needed from C(0); loads behind the later A chunks
                for sl in range(NSLAB):
                    ks = slice(sl * KSLAB, (sl + 1) * KSLAB)
                    nc.sync.dma_start(out=wo_sb[:, ks, :], in_=wo_d[:, ks, :])

                emit(b_steps(0), a_units(1))
                emit(b_steps(1), a_units(2))
                emit(b_steps(2), a_units(3))
                emit(b_steps(3), list(c_steps(0)) + list(c_steps(1)))
                # trailing C(2) covers AllGather(3)'s latency; C(3) then
                # starts dense
                emit(None, c_steps(2))
                emit(None, c_steps(3))
    nc.compile()
    return nc


def _host_consts():
    ident = np.eye(TB, dtype=np.float16)
    onesc = np.ones((TB, 1), dtype=np.float16)
    onesr = np.ones((1, DH), dtype=np.float32)
    tloc = np.arange(TB)[:, None]
    qloc = np.arange(TB)[None, :]
    tri = np.where(tloc <= qloc, 0.0, NEG_BIAS).astype(np.float16)
    return ident, onesc, onesr, tri


def _swizzle(w):
    # [D, C] -> [128, NKT, C] with element (p, kt, c) = w[kt*128 + p, c]
    return np.ascontiguousarray(
        w.reshape(NKT, 128, w.shape[1]).transpose(1, 0, 2)).astype(np.float16)


def kernel(x, mask, cos, sin, Wq, Wk, Wv, Wo):
    from concourse.bass_utils import run_bass_kernel_spmd

    if "nc" not in _CACHE:
        _CACHE["nc"] = _build_program()
    nc = _CACHE["nc"]

    x = np.asarray(x, dtype=np.float32)
    cos = np.asarray(cos, dtype=np.float32)
    sin = np.asarray(sin, dtype=np.float32)
    Wq = np.asarray(Wq, dtype=np.float32)
    Wk = np.asarray(Wk, dtype=np.float32)
    Wv = np.asarray(Wv, dtype=np.float32)
    Wo = np.asarray(Wo, dtype=np.float32)

    xTr = _swizzle(np.ascontiguousarray(x[0].T))       # [128, NKT, S]
    cosT = np.ascontiguousarray(cos.T).astype(np.float16)
    sinM = np.ascontiguousarray(sin.T).astype(np.float16)
    sinM[:DH // 2] = -sinM[:DH // 2]
    ident, onesc, onesr, tri = _host_consts()

    in_maps = []
    for i in range(N_CORES):
        in_maps.append({
            "xTr": xTr,
            "wq": _swizzle(Wq[:, i * FPC:(i + 1) * FPC]),
            "wk": _swizzle(Wk[:, i * DH:(i + 1) * DH]),
            "wv": _swizzle(Wv[:, i * DH:(i + 1) * DH]),
            "wo": _swizzle(Wo[:, i * FPC:(i + 1) * FPC]),
            "cosT": cosT,
            "sinM": sinM,
            "tri": tri,
            "ident": ident,
            "onesc": onesc,
            "onesr": onesr,
        })

    import os
    trace = bool(os.environ.get("BASS_TRACE"))
    res = run_bass_kernel_spmd(nc, in_maps, list(range(N_CORES)), trace=trace)
    _CACHE["last_exec_time_ns"] = res.exec_time_ns

    out = np.concatenate([res.results[i]["out"] for i in range(N_CORES)],
                         axis=1)
    return out[None]


# revision 39
# speedup vs baseline: 1.0627x; 1.0559x over previous
"""GQA attention (S=2048, D=4096, H=32, G=8, DH=128) on 8 trn2 cores.

Sharding: core i owns query heads [4i, 4i+4) and KV group i (column shards
of Wq/Wk/Wv). After attention each core holds a normalized context slice
ctx_loc [128, 4, 512] ([dh, head, query]); a per-chunk AllGather assembles
the full context and each core computes its 512-column shard of the output
projection. The host concatenates the 8 column shards.

All activations are feature-major ([feature, seq]):
  qT_h = Wq_h^T @ x^T           (PE, accumulate over D tiles)
  RoPE: the half-swap runs as two SBUF->SBUF DMAs against a host-negated
        sin table (no PE matmul, no extra PSUM bank)
  s[t,q] block = kT_tile.T @ qT chunk      (scoresT layout)
  p    = exp(s/sqrt(DH) - 4)    (ACT; bias keeps p in fp16 range)
  den  = running DVE sum of p tiles; 1/den via DVE fast reciprocal
  ctxT = v_block.T @ p          (PE accumulate)
  out  = ctx_tile.T @ Wo_shard  (PE, per-chunk after its AllGather)

Schedule: the whole kernel is one dense PE stream. Phase B (attention,
scalar-engine heavy) is interleaved INTO phase A's projection matmuls of
the next chunk, and phase C's output-projection matmuls fill phase B's
exp-latency gaps in the late iterations:
    A(0) | A(1)+B(0) | A(2)+B(1) | A(3)+B(2) | B(3)+C(0)+C(1) | C(2)+C(3)
A PE idle window >3.4us re-throttles the PE clock to 1.2 GHz, so density
is worth ~2x on its own. PSUM budget (8 banks): 6 projection accumulators
(reused by phase C's output accumulators via the same tag) + 1 score bank
+ 1 ctx bank. All HBM traffic moves in ~1MB slabs from host-preswizzled
[128, kt, col] layouts (the sync queue serializes dma_starts at ~0.6us
each, so small DMAs are poison).
"""

import math
import sys

if "/opt/trn_rl_repo" not in sys.path:
    sys.path.insert(0, "/opt/trn_rl_repo")

import numpy as np

S, D, H, G, DH = 2048, 4096, 32, 8, 128
N_CORES = 8
HPC = H // N_CORES          # query heads per core (4)
FPC = HPC * DH              # context features per core (512)
QC = 512                    # query chunk (matmul free dim)
NQC = S // QC               # 4
TB = 128                    # key block
NTB = S // TB               # 16
NKT = D // 128              # contraction tiles over D (32)
NJ = QC // TB               # key blocks per query chunk (4)
KSLAB = 8                   # kt tiles per x DMA slab
NSLAB = NKT // KSLAB        # 4
INV_SQRT_DH = 1.0 / math.sqrt(DH)
EXP_BIAS = -4.0             # keeps exp() outputs inside fp16 range
NEG_BIAS = -60000.0         # fp16-representable; exp() underflows to 0

_CACHE = {}


def _build_program():
    import concourse.mybir as mybir
    import concourse.tile as tile
    from concourse import bacc

    f32 = mybir.dt.float32
    f16 = mybir.dt.float16
    EXP = mybir.ActivationFunctionType.Exp

    nc = bacc.Bacc("TRN2", target_bir_lowering=False, debug=False,
                   num_devices=N_CORES)

    # host-preswizzled layouts: [128, kt, col] so each DMA is one 3D slab
    xTr_d = nc.dram_tensor("xTr", [128, NKT, S], f16, kind="ExternalInput")
    wq_d = nc.dram_tensor("wq", [128, NKT, FPC], f16, kind="ExternalInput")
    wk_d = nc.dram_tensor("wk", [128, NKT, DH], f16, kind="ExternalInput")
    wv_d = nc.dram_tensor("wv", [128, NKT, DH], f16, kind="ExternalInput")
    wo_d = nc.dram_tensor("wo", [128, NKT, FPC], f16, kind="ExternalInput")
    cosT_d = nc.dram_tensor("cosT", [DH, S], f16, kind="ExternalInput")
    # sinM = sin with rows [0, DH/2) negated: rotate_half(q)*sin == qswap*sinM
    sinM_d = nc.dram_tensor("sinM", [DH, S], f16, kind="ExternalInput")
    # triangle mask for the one diagonal 128x128 sub-block of each key block
    tri_d = nc.dram_tensor("tri", [TB, TB], f16, kind="ExternalInput")
    ident_d = nc.dram_tensor("ident", [TB, TB], f16, kind="ExternalInput")
    onesc_d = nc.dram_tensor("onesc", [TB, 1], f16, kind="ExternalInput")
    onesr_d = nc.dram_tensor("onesr", [1, DH], f32, kind="ExternalInput")
    out_d = nc.dram_tensor("out", [S, FPC], f32, kind="ExternalOutput")

    with tile.TileContext(nc) as tc:
        with tc.tile_pool(name="dram", bufs=1, space="DRAM") as dram:
            ctx_loc = [dram.tile([128, HPC, QC], f16, name=f"ctx_loc{qc}",
                                 tag=f"cl{qc}") for qc in range(NQC)]
            ctx_all = [dram.tile([N_CORES, 128, HPC, QC], f16,
                                 name=f"ctx_all{qc}", tag=f"ca{qc}",
                                 addr_space="Shared") for qc in range(NQC)]
            # chunk 3 gathers per head so each pass's AllGather starts at its
            # pass end instead of after the whole chunk
            cl3h = [dram.tile([128, QC], f16, name=f"cl3h{h}", tag=f"cl3h{h}")
                    for h in range(HPC)]
            ca3h = [dram.tile([N_CORES, 128, QC], f16, name=f"ca3h{h}",
                              tag=f"ca3h{h}", addr_space="Shared")
                    for h in range(HPC)]

            with tc.tile_pool(name="res", bufs=1) as res, \
                 tc.tile_pool(name="str", bufs=1) as pS, \
                 tc.tile_pool(name="ps", bufs=1, space="PSUM") as ps:

                # tiles for tiny consts; their DMAs are emitted inside the
                # second A(0) unit so the critical wk/x/wq loads go first
                ident_sb = res.tile([TB, TB], f16, tag="ident", name="ident_sb")
                onesc_sb = res.tile([TB, 1], f16, tag="onesc", name="onesc_sb")
                onesr_sb = res.tile([1, DH], f32, tag="onesr", name="onesr_sb")
                ebias_sb = res.tile([128, 1], f32, tag="ebias", name="ebias_sb")
                nc.vector.memset(ebias_sb[:], EXP_BIAS)
                tri_sb = res.tile([TB, TB], f16, tag="tri", name="tri_sb")

                wk_sb = res.tile([128, NKT, DH], f16, tag="wk", name="wk_sb")
                nc.sync.dma_start(out=wk_sb[:], in_=wk_d[:])
                wv_sb = res.tile([128, NKT, DH], f16, tag="wv", name="wv_sb")
                nc.sync.dma_start(out=wv_sb[:], in_=wv_d[:])
                wq_sb = res.tile([128, NKT, FPC], f16, tag="wq", name="wq_sb")
                wo_sb = res.tile([128, NKT, FPC], f16, tag="wo", name="wo_sb")

                # per-chunk activation tiles (separate tiles so cross-chunk
                # writer/reader deps stay slice-exact)
                qT_sb = [[res.tile([128, QC], f16, tag=f"qT{h}_{c}",
                                   name=f"qT{h}_{c}") for c in range(NQC)]
                         for h in range(HPC)]
                kT_sb = [res.tile([128, QC], f16, tag=f"kT{c}", name=f"kT{c}")
                         for c in range(NQC)]
                v_sb = [res.tile([128, NJ, TB], f16, tag=f"v{c}",
                                 name=f"v{c}") for c in range(NQC)]

                # ============ phase A unit generator (one chunk) ===========
                def a_units(c):
                    """Yield closures; each emits 6 matmuls (one kt across
                    the 6 projections). Final units emit rope + v-evict."""
                    csl = slice(c * QC, (c + 1) * QC)
                    cos_c = pS.tile([DH, QC], f16, tag="cosc", bufs=2,
                                    name="cos_c")
                    sin_c = pS.tile([DH, QC], f16, tag="sinc", bufs=2,
                                    name="sin_c")
                    if c > 0:
                        nc.sync.dma_start(out=cos_c[:], in_=cosT_d[:, csl])
                        nc.sync.dma_start(out=sin_c[:], in_=sinM_d[:, csl])
                    k_ps = ps.tile([128, QC], f32, tag="acc", bufs=6,
                                   name="k_ps")
                    vT_ps = ps.tile([128, QC], f32, tag="acc", bufs=6,
                                    name="vT_ps")
                    q_ps = [ps.tile([128, QC], f32, tag="acc", bufs=6,
                                    name=f"q_ps{h}") for h in range(HPC)]
                    xt = [None]

                    def unit(kt):
                        sl, k = divmod(kt, KSLAB)
                        if k == 0:
                            xt[0] = pS.tile([128, KSLAB, QC], f16, tag="xs",
                                            bufs=2, name="xt")
                            nc.sync.dma_start(
                                out=xt[0][:],
                                in_=xTr_d[:, sl * KSLAB:(sl + 1) * KSLAB,
                                          csl])
                            if c == 0:
                                ks = slice(sl * KSLAB, (sl + 1) * KSLAB)
                                nc.sync.dma_start(out=wq_sb[:, ks, :],
                                                  in_=wq_d[:, ks, :])
                            if c == 0 and sl == 0:
                                nc.sync.dma_start(out=cos_c[:],
                                                  in_=cosT_d[:, csl])
                                nc.sync.dma_start(out=sin_c[:],
                                                  in_=sinM_d[:, csl])
                        if c == 0 and kt == 1:
                            nc.sync.dma_start(out=ident_sb[:], in_=ident_d[:])
                            nc.sync.dma_start(out=onesc_sb[:], in_=onesc_d[:])
                            nc.sync.dma_start(out=onesr_sb[:], in_=onesr_d[:])
                            nc.sync.dma_start(out=tri_sb[:], in_=tri_d[:])
                        st, sp = kt == 0, kt == NKT - 1
                        xk = xt[0][:, k, :]
                        nc.tensor.matmul(k_ps[:], wk_sb[:, kt, :], xk,
                                         start=st, stop=sp)
                        nc.tensor.matmul(vT_ps[:], wv_sb[:, kt, :], xk,
                                         start=st, stop=sp)
                        for h in range(HPC):
                            nc.tensor.matmul(q_ps[h][:],
                                             wq_sb[:, kt, h * DH:(h + 1) * DH],
                                             xk, start=st, stop=sp)

                    def rope(src_ps, dst_ap):
                        qc_sb = pS.tile([128, QC], f16, tag="ropecp", bufs=2,
                                        name="qc_sb")
                        nc.scalar.copy(qc_sb[:], src_ps[:])
                        qsw = pS.tile([128, QC], f16, tag="ropesw", bufs=2,
                                      name="qsw")
                        hf = DH // 2
                        nc.sync.dma_start(out=qsw[0:hf, :],
                                            in_=qc_sb[hf:DH, :])
                        nc.sync.dma_start(out=qsw[hf:DH, :],
                                            in_=qc_sb[0:hf, :])
                        t1 = pS.tile([128, QC], f16, tag="ropet1", bufs=2,
                                     name="t1")
                        nc.vector.tensor_mul(t1[:], qsw[:], sin_c[:])
                        nc.vector.tensor_mul(dst_ap, qc_sb[:], cos_c[:])
                        nc.vector.tensor_add(dst_ap, dst_ap, t1[:])

                    def tail_k():
                        rope(k_ps, kT_sb[c][:])

                    def tail_v():
                        vts = pS.tile([128, QC], f16, tag="vts", bufs=2,
                                      name="vts")
                        nc.scalar.copy(vts[:], vT_ps[:])
                        for sb in range(NJ):
                            tr_ps = ps.tile([TB, TB], f16, tag="s", bufs=1,
                                            name="tr_ps")
                            nc.tensor.transpose(tr_ps[:],
                                                vts[:, sb * TB:(sb + 1) * TB],
                                                ident_sb[:])
                            nc.scalar.copy(v_sb[c][:, sb, :], tr_ps[:])

                    for kt in range(NKT):
                        yield lambda kt=kt: unit(kt)
                    yield tail_k
                    yield tail_v
                    for h in range(HPC):
                        yield lambda h=h: rope(q_ps[h], qT_sb[h][c][:])

                # ================== phase B (one chunk) ====================
                def b_steps(qcn):
                    """Yield (step, kind) closures: single-head passes over
                    the key blocks; each block step takes a filler callable
                    run between its score and ctx matmuls."""
                    ntb = (qcn + 1) * NJ

                    def make_pass(h):
                        den = pS.tile([128, QC], f32, tag="den", bufs=2,
                                      name="den")
                        ctx_ps = ps.tile([128, QC], f32, tag="ctx", bufs=1,
                                         name="ctx_ps")
                        dr = [None]

                        def block(tb, filler):
                            j = tb - qcn * NJ
                            # diagonal key blocks: queries < j*TB are fully
                            # masked — skip their columns entirely
                            q0 = max(j, 0) * TB
                            w = QC - q0
                            s_ps = ps.tile([128, QC], f32, tag="s", bufs=1,
                                           name="s_ps")
                            nc.tensor.matmul(
                                s_ps[:, :w],
                                kT_sb[tb // NJ][:, (tb % NJ) * TB:
                                                (tb % NJ + 1) * TB],
                                qT_sb[h][qcn][:, q0:], start=True, stop=True)
                            if j >= 0:
                                # only the leading TB columns of the live
                                # range form the triangle
                                nc.vector.tensor_add(s_ps[:, :TB],
                                                     s_ps[:, :TB], tri_sb[:])
                            p_sb = pS.tile([128, QC], f16, tag="p", bufs=8,
                                           name="p_sb")
                            nc.scalar.activation(p_sb[:, :w], s_ps[:, :w],
                                                 EXP, bias=ebias_sb[:],
                                                 scale=INV_SQRT_DH)
                            if tb == 0:
                                nc.vector.tensor_copy(den[:], p_sb[:])
                            elif tb == ntb - 1:
                                # last block is diagonal j=NJ-1 (w == TB)
                                d = pS.tile([128, QC], f16, tag="dr", bufs=2,
                                            name="dr")
                                nc.vector.tensor_copy(d[:, :q0], den[:, :q0])
                                nc.vector.tensor_add(d[:, q0:], den[:, q0:],
                                                     p_sb[:, :w])
                                dr[0] = d
                            else:
                                nc.vector.tensor_add(den[:, q0:], den[:, q0:],
                                                     p_sb[:, :w])
                            filler()
                            nc.tensor.matmul(ctx_ps[:, q0:],
                                             v_sb[tb // NJ][:, tb % NJ, :],
                                             p_sb[:, :w], start=(tb == 0),
                                             stop=(tb == ntb - 1))

                        def normalize(filler):
                            aux1 = ps.tile([128, QC], f32, tag="s", bufs=1,
                                           name="aux1")
                            nc.tensor.matmul(aux1[:1, :], onesc_sb[:],
                                             dr[0][:], start=True, stop=True)
                            recf = pS.tile([1, QC], f32, tag="recf", bufs=2,
                                           name="recf")
                            nc.vector.reciprocal_approx_fast(out=recf[:],
                                                             in_=aux1[:1, :])
                            filler()
                            aux2 = ps.tile([128, QC], f32, tag="s", bufs=1,
                                           name="aux2")
                            nc.tensor.matmul(aux2[:], onesr_sb[:], recf[:],
                                             start=True, stop=True)
                            rb = pS.tile([128, QC], f16, tag="rb", bufs=2,
                                         name="rb")
                            nc.vector.tensor_copy(rb[:], aux2[:])
                            ctx_sb = pS.tile([128, QC], f16, tag="ctxsb",
                                             bufs=2, name="ctx_sb")
                            nc.vector.tensor_mul(ctx_sb[:], ctx_ps[:], rb[:])
                            # scalar-queue DMA: the sync queue gets dammed
                            # behind the previous AllGather (ring-hazard
                            # wait), which would delay this write and with it
                            # the next AllGather's trigger
                            if qcn == NQC - 1:
                                nc.scalar.dma_start(out=cl3h[h][:],
                                                    in_=ctx_sb[:])
                                nc.gpsimd.collective_compute(
                                    "AllGather", mybir.AluOpType.bypass,
                                    replica_groups=[list(range(N_CORES))],
                                    ins=[cl3h[h].opt()],
                                    outs=[ca3h[h].opt()])
                            else:
                                nc.scalar.dma_start(out=ctx_loc[qcn][:, h, :],
                                                    in_=ctx_sb[:])

                        for tb in range(ntb):
                            yield lambda filler, tb=tb: block(tb, filler)
                        yield normalize

                    for h in range(HPC):
                        yield from make_pass(h)

                    def trigger(filler):
                        filler()
                        if qcn != NQC - 1:
                            nc.gpsimd.collective_compute(
                                "AllGather", mybir.AluOpType.bypass,
                                replica_groups=[list(range(N_CORES))],
                                ins=[ctx_loc[qcn].opt()],
                                outs=[ctx_all[qcn].opt()])
                    yield trigger

                # ================== phase C (one chunk) ====================
                def c_steps(qcn):
                    ct = [None] * N_CORES

                    def load_half(half):
                        for i in range(N_CORES):
                            t = pS.tile([128, HPC, QC // 2], f16, tag="ct",
                                        bufs=9, name="ct")
                            nc.sync.dma_start(
                                out=t[:],
                                in_=ctx_all[qcn][i][:, :,
                                                    half * (QC // 2):
                                                    (half + 1) * (QC // 2)])
                            ct[i] = t

                    o_ps = [None]
                    o_cnt = [0]

                    def mm_run(qb, i0):
                        if o_cnt[0] == 0:
                            o_ps[0] = ps.tile([TB, FPC], f32, tag="acc",
                                              bufs=6, name="o_ps")
                        qoff = (qb % 2) * TB
                        for i in (i0, i0 + 1):
                            for jj in range(HPC):
                                kt = i * HPC + jj
                                nc.tensor.matmul(
                                    o_ps[0][:], ct[i][:, jj, qoff:qoff + TB],
                                    wo_sb[:, kt, :], start=(kt == 0),
                                    stop=(kt == NKT - 1))
                        o_cnt[0] += 2
                        if o_cnt[0] == N_CORES:
                            o_cnt[0] = 0
                            o_sb = pS.tile([TB, FPC], f32, tag="osb", bufs=2,
                                           name="o_sb")
                            nc.vector.tensor_copy(o_sb[:], o_ps[0][:])
                            qrow = qcn * QC + qb * TB
                            nc.sync.dma_start(out=out_d[qrow:qrow + TB, :],
                                                in_=o_sb[:])

                    for qb in range(NJ):
                        if qb % 2 == 0:
                            yield lambda h=qb // 2: load_half(h)
                        for i0 in (0, 2, 4, 6):
                            yield lambda qb=qb, i0=i0: mm_run(qb, i0)

                # =================== interleaved emission ==================
                def emit(b_gen, fill_steps):
                    """Emit B steps, injecting filler closures into the
                    exp-latency slots, spread evenly (exact Bresenham)."""
                    fill = list(fill_steps)
                    bs = list(b_gen) if b_gen is not None else []
                    fi = [0]
                    nf, nb = len(fill), len(bs)

                    def filler_n(n):
                        def f():
                            for _ in range(n):
                                if fi[0] < nf:
                                    fill[fi[0]]()
                                    fi[0] += 1
                        return f

                    for bi, bstep in enumerate(bs):
                        n = (bi + 1) * nf // nb - bi * nf // nb
                        bstep(filler_n(n))
                    while fi[0] < nf:
                        fill[fi[0]]()
                        fi[0] += 1

                def as_fill(units):
                    # adapt no-arg closures to filler-taking b-steps
                    return [(lambda f, u=u: (u(), f())) for u in units]

                def c3_steps():
                    """Chunk-3 output projection over the per-head gathers,
                    head-major so heads 0-2 process while head 3 gathers.
                    Four PSUM accumulators (one per query block), one ct
                    tile live at a time."""
                    o_ps = [ps.tile([TB, FPC], f32, tag="acc", bufs=6,
                                    name=f"o3_{qb}") for qb in range(NJ)]
                    ct3 = [None, None]

                    def step(i):
                        if i < HPC * N_CORES:
                            h, core = divmod(i, N_CORES)
                            t = pS.tile([128, QC], f16, tag="ct3", bufs=3,
                                        name="ct3")
                            nc.sync.dma_start(out=t[:], in_=ca3h[h][core])
                            ct3[i % 2] = (t, h, core)
                        if i > 0:
                            t, h, core = ct3[(i - 1) % 2]
                            kt = core * HPC + h
                            st = h == 0 and core == 0
                            sp = h == HPC - 1 and core == N_CORES - 1
                            for qb in range(NJ):
                                nc.tensor.matmul(
                                    o_ps[qb][:],
                                    t[:, qb * TB:(qb + 1) * TB],
                                    wo_sb[:, kt, :], start=st, stop=sp)
                        if i == HPC * N_CORES:
                            for qb in range(NJ):
                                o_sb = pS.tile([TB, FPC], f32, tag="osb",
                                               bufs=2, name="o_sb")
                                nc.vector.tensor_copy(o_sb[:], o_ps[qb][:])
                                qrow = (NQC - 1) * QC + qb * TB
                                nc.sync.dma_start(
                                    out=out_d[qrow:qrow + TB, :], in_=o_sb[:])

                    for i in range(HPC * N_CORES + 1):
                        yield lambda i=i: step(i)

                # A(0) runs alone (nothing to overlap yet)
                emit(as_fill(a_units(0)), [])
                # wo needed from C(0); loads behind the later A chunks
                for sl in range(NSLAB):
                    ks = slice(sl * KSLAB, (sl + 1) * KSLAB)
                    nc.sync.dma_start(out=wo_sb[:, ks, :], in_=wo_d[:, ks, :])

                emit(b_steps(0), a_units(1))
                emit(b_steps(1), a_units(2))
                emit(b_steps(2), a_units(3))
                emit(b_steps(3), list(c_steps(0)) + list(c_steps(1)))
                # trailing C(2) covers the tail of chunk 3's per-head
                # gathers; C(3) then consumes them head-major
                emit(None, c_steps(2))
                emit(None, c3_steps())
    nc.compile()
    return nc


def _host_consts():
    ident = np.eye(TB, dtype=np.float16)
    onesc = np.ones((TB, 1), dtype=np.float16)
    onesr = np.ones((1, DH), dtype=np.float32)
    tloc = np.arange(TB)[:, None]
    qloc = np.arange(TB)[None, :]
    tri = np.where(tloc <= qloc, 0.0, NEG_BIAS).astype(np.float16)
    return ident, onesc, onesr, tri


def _swizzle(w):
    # [D, C] -> [128, NKT, C] with element (p, kt, c) = w[kt*128 + p, c]
    return np.ascontiguousarray(
        w.reshape(NKT, 128, w.shape[1]).transpose(1, 0, 2)).astype(np.float16)


def kernel(x, mask, cos, sin, Wq, Wk, Wv, Wo):
    from concourse.bass_utils import run_bass_kernel_spmd

    if "nc" not in _CACHE:
        _CACHE["nc"] = _build_program()
    nc = _CACHE["nc"]

    x = np.asarray(x, dtype=np.float32)
    cos = np.asarray(cos, dtype=np.float32)
    sin = np.asarray(sin, dtype=np.float32)
    Wq = np.asarray(Wq, dtype=np.float32)
    Wk = np.asarray(Wk, dtype=np.float32)
    Wv = np.asarray(Wv, dtype=np.float32)
    Wo = np.asarray(Wo, dtype=np.float32)

    xTr = _swizzle(np.ascontiguousarray(x[0].T))       # [128, NKT, S]
    cosT = np.ascontiguousarray(cos.T).astype(np.float16)
    sinM = np.ascontiguousarray(sin.T).astype(np.float16)
    sinM[:DH // 2] = -sinM[:DH // 2]
    ident, onesc, onesr, tri = _host_consts()

    in_maps = []
    for i in range(N_CORES):
        in_maps.append({
            "xTr": xTr,
            "wq": _swizzle(Wq[:, i * FPC:(i + 1) * FPC]),
            "wk": _swizzle(Wk[:, i * DH:(i + 1) * DH]),
            "wv": _swizzle(Wv[:, i * DH:(i + 1) * DH]),
            "wo": _swizzle(Wo[:, i * FPC:(i + 1) * FPC]),
            "cosT": cosT,
            "sinM": sinM,
            "tri": tri,
            "ident": ident,
            "onesc": onesc,
            "onesr": onesr,
        })

    import os
    trace = bool(os.environ.get("BASS_TRACE"))
    res = run_bass_kernel_spmd(nc, in_maps, list(range(N_CORES)), trace=trace)
    _CACHE["last_exec_time_ns"] = res.exec_time_ns

    out = np.concatenate([res.results[i]["out"] for i in range(N_CORES)],
                         axis=1)
    return out[None]
